# revision 26
# baseline (speedup 1.0000x reference)
"""DiffSAGE GNN layer on 8 Trainium2 NeuronCores.

Math (per reference):
    msg      = x[src] - x[dst]                      # per edge
    agg      = segment_mean(msg, dst, N)            # zeros where cnt==0
    out      = agg @ Wl.T + bl + x @ Wr.T

Identities used:
    sum_{e: dst=i} (x[src_e] - x[i]) = gsum[i] - cnt[i]*x[i]
    agg[i] = gsum[i]*r[i] - s[i]*x[i],  r = 1/max(cnt,1), s = cnt*r
    out    = (r ⊙ gsum) @ Wl.T + x @ (Wr - s⊙Wl).T + bl
For nodes with cnt>0, s=1, so with W2 = Wr - Wl (host-computed):
    out_row[n] = r[n] * (gsum @ Wl.T)[n] + (x @ W2.T)[n] + bl
cnt==0 nodes (rare to nonexistent) are patched exactly on the host.

Distribution: destination-node sharding across the 8 cores (core c owns node
rows [c*N/8, (c+1)*N/8)).  Each core dma-gathers x[src] rows for its own
edges straight from HBM; no collectives.

Device algorithm per core (single compiled SPMD program, data-uniform):
  - nodes are cut into 128-node windows; per (window, side) the edge count is
    padded to a 128-slot chunk multiple with capacity = max over the 8 cores
    (so the instruction stream is uniform).  Sides: A = src < 32768, B =
    src >= 32768 (dma_gather indices are int16).
  - gathers run on 4 SWDGE queues, several calls in flight (the gather is
    descriptor-rate-bound at ~2ns/row; queue parallelism is the lever).
    Trailing pad slots are marked -1 (skipped by HW).
  - per 128-edge chunk: one-hot[e, n] = (dst_rel[e] == iota[n]) on DVE, then
    PE accumulates psum_g[feat, node] += msg_chunk.T @ one-hot.
  - window epilogue (no transposes):
      z1[n, dout] = matmul(lhsT=gsumT_sb, rhs=Wl.T)   # lhsT free dim -> out
      v[n, dout]  = matmul(lhsT=xT_w, rhs=W2.T) + ones⊗bl
      out_rows    = r[n] * z1 + v        (tensor_scalar + tensor_tensor)
    and a contiguous [128, 128] f32 store.
"""

import os
import sys

import numpy as np

try:
    import concourse.bass as bass
except Exception:  # pragma: no cover - harness path setup
    for p in (
        "/root/.axon_site",
        "/root/.axon_site/_ro/trn_rl_repo",
        "/root/.axon_site/_ro/pypackages",
        "/opt/trn_rl_repo",
    ):
        if p not in sys.path:
            sys.path.append(p)
    import concourse.bass as bass

from contextlib import ExitStack

import ml_dtypes

import concourse.mybir as mybir
import concourse.tile as tile
from concourse import bacc, bass_utils

F32 = mybir.dt.float32
BF16 = mybir.dt.bfloat16
I16 = mybir.dt.int16
I32 = mybir.dt.int32

D = 128          # feature dim (in and out)
WN = 128         # nodes per window
CHUNK = 128      # edges per matmul chunk (contraction dim)
SPLIT = 32768    # x rows < SPLIT go to table A, rest to table B
GROUP = 2        # windows whose gathers are batched into one call pair
NQ = 4           # SWDGE queues
STAGE_BUFS = 5   # gather staging depth (per side), in window-groups


class Cfg:
    def __init__(self, nta, ntb, wins, capa, capb, n_cores, group=GROUP):
        self.NTA = nta              # rows in table A
        self.NTB = ntb              # rows in table B
        self.WINS = wins            # 128-node windows per core
        self.CAPA = capa            # per-window A chunk capacities [wins]
        self.CAPB = capb            # per-window B chunk capacities [wins]
        self.NSLAB = wins * WN      # padded nodes per core
        self.N_CORES = n_cores
        self.G = group
        assert wins % group == 0
        caps = [capa[w] + capb[w] for w in range(wins)]
        self.NCHUNKS = int(sum(caps))       # total matmul chunks
        self.TOT16 = self.NCHUNKS * 8       # idx cols (128 slots = 8 cols)


def build_nc(cfg: Cfg, repeat: int = 1, nq: int = NQ,
             skip_gather: bool = False, skip_chunks: bool = False,
             stage_bufs: int = STAGE_BUFS) -> bass.Bass:
    nc = bacc.Bacc("TRN2", num_swdge_queues=nq)
    WINS, CAPA, CAPB, NSLAB = cfg.WINS, cfg.CAPA, cfg.CAPB, cfg.NSLAB
    G = cfg.G

    taba = nc.dram_tensor("taba", [cfg.NTA, D], BF16, kind="ExternalInput")
    tabb = nc.dram_tensor("tabb", [cfg.NTB, D], BF16, kind="ExternalInput")
    idxh = nc.dram_tensor("idx", [128, cfg.TOT16], I16, kind="ExternalInput")
    dsth = nc.dram_tensor("dstr", [128, cfg.NCHUNKS], BF16, kind="ExternalInput")
    xts = nc.dram_tensor("xts", [D, NSLAB], BF16, kind="ExternalInput")
    rh = nc.dram_tensor("r", [128, WINS], F32, kind="ExternalInput")
    wlt = nc.dram_tensor("wlt", [D, D], F32, kind="ExternalInput")
    w2t = nc.dram_tensor("w2t", [D, D], BF16, kind="ExternalInput")
    blb = nc.dram_tensor("blb", [1, D], F32, kind="ExternalInput")
    outh = nc.dram_tensor("out", [NSLAB, D], F32, kind="ExternalOutput")

    # static per-window offsets (uniform across cores)
    chunk_off = np.zeros(WINS + 1, dtype=np.int64)
    for w in range(WINS):
        chunk_off[w + 1] = chunk_off[w] + CAPA[w] + CAPB[w]

    with ExitStack() as ctx:
        tc = ctx.enter_context(tile.TileContext(nc))
        singles = ctx.enter_context(tc.tile_pool(name="singles", bufs=1))
        stage_p = ctx.enter_context(tc.tile_pool(name="stage", bufs=stage_bufs))
        oh_p = ctx.enter_context(tc.tile_pool(name="oh", bufs=4))
        wrk = ctx.enter_context(tc.tile_pool(name="wrk", bufs=3))
        pacc = ctx.enter_context(tc.tile_pool(name="pacc", bufs=2, space="PSUM"))
        pepi = ctx.enter_context(tc.tile_pool(name="pepi", bufs=2, space="PSUM"))

        # ---- one-time constants ----
        xt_sb = singles.tile([D, NSLAB], BF16)
        nc.sync.dma_start(out=xt_sb[:], in_=xts[:])
        idx_sb = singles.tile([128, cfg.TOT16], I16)
        nc.sync.dma_start(out=idx_sb[:], in_=idxh[:])
        dst_sb = singles.tile([128, cfg.NCHUNKS], BF16)
        nc.sync.dma_start(out=dst_sb[:], in_=dsth[:])
        r_sb = singles.tile([128, WINS], F32)
        nc.sync.dma_start(out=r_sb[:], in_=rh[:])
        wlt_sb = singles.tile([D, D], F32)
        nc.sync.dma_start(out=wlt_sb[:], in_=wlt[:])
        w2t_sb = singles.tile([D, D], BF16)
        nc.sync.dma_start(out=w2t_sb[:], in_=w2t[:])
        bl_sb = singles.tile([1, D], F32)
        nc.sync.dma_start(out=bl_sb[:], in_=blb[:])
        ones1 = singles.tile([1, WN], F32)
        nc.vector.memset(ones1[:], 1.0)
        iota_i = singles.tile([CHUNK, WN], I32)
        nc.gpsimd.iota(iota_i[:], pattern=[[1, WN]], channel_multiplier=0)
        iota_b = singles.tile([CHUNK, WN], BF16)
        nc.vector.tensor_copy(out=iota_b[:], in_=iota_i[:])

        state = {}
        qload = [0] * nq  # greedy static balance of descriptors per queue

        def next_q(ndesc):
            q = qload.index(min(qload))
            qload[q] += ndesc
            return q

        def window_body(w):
            g, wg = divmod(w, G)
            if wg == 0:
                # issue the gathers for this window-group (one call per side)
                ws = list(range(g * G, (g + 1) * G))
                ca = [CAPA[x] for x in ws]
                cb = [CAPB[x] for x in ws]
                na, nb = sum(ca), sum(cb)
                # idx layout per group: A chunks of all windows, then B chunks

                def gather_side(tab, nch, i16, tag):
                    stg = stage_p.tile([CHUNK, nch, D], BF16, tag=tag)
                    if skip_gather:
                        nc.vector.memset(stg[:, 0, :], 0)
                        return stg
                    # split into two half-calls on different queues: doubles
                    # the number of in-flight gathers (descriptor-rate-bound)
                    h = (nch + 1) // 2
                    for lo, hi in ((0, h), (h, nch)):
                        if hi <= lo:
                            continue
                        nc.gpsimd.dma_gather(
                            stg[:, lo:hi, :], tab,
                            idx_sb[:, i16 + lo * 8 : i16 + hi * 8],
                            (hi - lo) * CHUNK, (hi - lo) * CHUNK, D,
                            single_packet=False,
                            queue_num=next_q(hi - lo),
                        )
                    return stg

                i16 = chunk_off[ws[0]] * 8
                if na:
                    state["sa"] = gather_side(taba[:], na, i16, "sa")
                if nb:
                    state["sb"] = gather_side(tabb[:], nb, i16 + na * 8, "sb")
                state["oa"] = 0
                state["ob"] = 0

            capa, capb = CAPA[w], CAPB[w]
            cap = capa + capb
            if cap == 0:
                return
            pg = pacc.tile([D, WN], F32, space="PSUM", tag="pg")
            # dstr columns for this window: A chunks then B chunks, matching
            # the staging slot order within the group tiles
            for c in ([0] if skip_chunks else range(cap)):
                if c < capa:
                    msg = state["sa"][:, state["oa"] + c, :]
                    col = chunk_off[w] + c
                else:
                    msg = state["sb"][:, state["ob"] + (c - capa), :]
                    col = chunk_off[w] + c
                oh = oh_p.tile([CHUNK, WN], BF16, tag="oh")
                nc.vector.tensor_tensor(
                    out=oh[:],
                    in0=dst_sb[:, col : col + 1].to_broadcast([CHUNK, WN]),
                    in1=iota_b[:],
                    op=mybir.AluOpType.is_equal,
                )
                nc.tensor.matmul(
                    pg[:], lhsT=msg, rhs=oh[:],
                    start=(c == 0),
                    stop=(c == cap - 1) or skip_chunks,
                )
            state["oa"] += capa
            state["ob"] += capb

            # ---- window epilogue ----
            gsum_sb = wrk.tile([D, WN], F32, tag="g")
            nc.vector.tensor_copy(out=gsum_sb[:], in_=pg[:])
            z1 = pepi.tile([WN, D], F32, space="PSUM", tag="z1")
            nc.tensor.matmul(z1[:], lhsT=gsum_sb[:], rhs=wlt_sb[:],
                             start=True, stop=True)
            v = pepi.tile([WN, D], F32, space="PSUM", tag="v")
            xw = xt_sb[:, w * WN : (w + 1) * WN]
            nc.tensor.matmul(v[:], lhsT=xw, rhs=w2t_sb[:],
                             start=True, stop=False)
            nc.tensor.matmul(v[:], lhsT=ones1[:], rhs=bl_sb[:],
                             start=False, stop=True)
            tmp = wrk.tile([WN, D], F32, tag="t")
            nc.vector.tensor_scalar_mul(tmp[:], z1[:], r_sb[:, w : w + 1])
            outt = wrk.tile([WN, D], F32, tag="o")
            nc.vector.tensor_tensor(
                out=outt[:], in0=tmp[:], in1=v[:], op=mybir.AluOpType.add
            )
            nc.sync.dma_start(out=outh[w * WN : (w + 1) * WN, :], in_=outt[:])

        if repeat > 1:
            rep_start = nc.snap(0)
            rep_end = nc.snap(repeat)
            with tc.For_i(rep_start, rep_end, 1, name="rep"):
                for w in range(WINS):
                    window_body(w)
        else:
            for w in range(WINS):
                window_body(w)

    nc.compile()
    return nc


def wrap_idx(idx):
    """[n] -> [128, n/16] int16 (16-partition wrap, replicated 8x)."""
    n = len(idx)
    w = idx.reshape(n // 16, 16)
    return np.ascontiguousarray(np.tile(w.T, (8, 1)))


def prep_core(src, rel, win, is_b, capa, capb, wins, group):
    """Build this core's flat idx (int64 pre-wrap; -1 only on call tails) and
    dstr ([nchunks, 128] f32, 999 pads) from its edges.

    Stream/consumption order: per window-group, one gather call per side,
    covering that side's chunks of all windows in the group in window order.
    """
    grp = win // group
    order = np.lexsort((src, win, is_b, grp))
    src, rel, win, is_b = src[order], rel[order], win[order], is_b[order]
    nchunks = int(np.sum(capa) + np.sum(capb))
    chunk_off = np.zeros(wins + 1, dtype=np.int64)
    chunk_off[1:] = np.cumsum(capa + capb)
    dstr = np.full((nchunks, CHUNK), 999.0, dtype=np.float32)
    counts_a = np.bincount(win[~is_b], minlength=wins)
    counts_b = np.bincount(win[is_b], minlength=wins)
    gidx = np.where(is_b, src - SPLIT, src).astype(np.int64)
    idx_parts = []
    pos = 0
    for g in range(wins // group):
        ws = list(range(g * group, (g + 1) * group))
        for side, counts, caps in (
            (0, counts_a, capa), (1, counts_b, capb),
        ):
            call = []
            for w in ws:
                cnt = int(counts[w])
                cap = int(caps[w])
                assert cnt <= cap * CHUNK
                if cap == 0:
                    continue
                # pads gather row 0 (real data; the 999 one-hot kills their
                # contribution).  -1 skip-indices wedge the device — don't.
                sl = np.zeros(cap * CHUNK, dtype=np.int64)
                sl[:cnt] = gidx[pos : pos + cnt]
                cbase = int(chunk_off[w]) + (int(capa[w]) if side else 0)
                dstr[cbase : cbase + cap].reshape(-1)[:cnt] = rel[
                    pos : pos + cnt
                ]
                call.append(sl)
                pos += cnt
            if call:
                idx_parts.append(np.concatenate(call))
    assert pos == len(src)
    idx_flat = (
        np.concatenate(idx_parts) if idx_parts else np.zeros(0, np.int64)
    )
    assert len(idx_flat) == nchunks * CHUNK
    return idx_flat, dstr


def run_graph(x, edge_index, Wl, bl, Wr, n_cores=8, group=GROUP, trace=False,
              repeat=1, nq=NQ, skip_gather=False, **bopts):
    """Full pipeline: host prep -> one SPMD compile -> run -> unshard."""
    x = np.asarray(x, dtype=np.float32)
    n, d = x.shape
    assert d == D
    src = np.asarray(edge_index[0], dtype=np.int64)
    dst = np.asarray(edge_index[1], dtype=np.int64)
    assert n % n_cores == 0
    npc = n // n_cores
    wins = -(-npc // WN)
    while wins % group:
        wins += 1
    nslab = wins * WN

    core_of = dst // npc
    ldst = dst - core_of * npc
    win_all = ldst // WN
    rel_all = (ldst % WN).astype(np.float32)
    is_b_all = src >= SPLIT

    # per-window capacities: max over cores (uniform SPMD program)
    wid = core_of * wins + win_all
    ca = np.bincount(wid[~is_b_all], minlength=n_cores * wins).reshape(
        n_cores, wins
    )
    cb = np.bincount(wid[is_b_all], minlength=n_cores * wins).reshape(
        n_cores, wins
    )
    capa = -(-ca.max(axis=0) // CHUNK)
    capb = -(-cb.max(axis=0) // CHUNK)

    nta = min(n, SPLIT)
    ntb = max(n - SPLIT, 1)
    cfg = Cfg(nta=nta, ntb=ntb, wins=wins, capa=capa, capb=capb,
              n_cores=n_cores, group=group)
    if os.environ.get("KERNEL_VERBOSE"):
        slots = cfg.NCHUNKS * CHUNK
        print(
            f"[kernel] wins={wins} chunks={cfg.NCHUNKS} slots={slots} "
            f"real={len(src) // n_cores} pads={slots - len(src) // n_cores} "
            f"({100 * (slots - len(src) / n_cores) / slots:.1f}%)"
        )

    taba = x[:nta].astype(ml_dtypes.bfloat16)
    tabb = (
        x[SPLIT:].astype(ml_dtypes.bfloat16)
        if n > SPLIT
        else np.zeros((1, D), dtype=ml_dtypes.bfloat16)
    )
    wlt_h = np.ascontiguousarray(np.asarray(Wl, np.float32).T)
    w2t_h = np.ascontiguousarray(
        (np.asarray(Wr, np.float32) - np.asarray(Wl, np.float32)).T
    ).astype(ml_dtypes.bfloat16)
    blb_h = np.ascontiguousarray(np.asarray(bl, np.float32).reshape(1, D))

    cnt_all = np.bincount(dst, minlength=n).astype(np.float32)
    r_all = 1.0 / np.maximum(cnt_all, 1.0)

    in_maps = []
    for c in range(n_cores):
        m = core_of == c
        idx_flat, dstr = prep_core(
            src[m], rel_all[m], win_all[m], is_b_all[m], capa, capb, wins,
            group,
        )
        idx_w = wrap_idx(idx_flat.astype(np.int16))
        # dstr: [nchunks, 128] -> [128, nchunks]
        dstr_t = np.ascontiguousarray(dstr.T.astype(ml_dtypes.bfloat16))
        xs = np.zeros((D, nslab), dtype=ml_dtypes.bfloat16)
        xs[:, :npc] = x[c * npc : (c + 1) * npc].T.astype(ml_dtypes.bfloat16)
        rs = np.zeros((wins, WN), dtype=np.float32)
        rs.reshape(-1)[:npc] = r_all[c * npc : (c + 1) * npc]
        in_maps.append(
            {
                "taba": taba,
                "tabb": tabb,
                "idx": idx_w,
                "dstr": dstr_t,
                "xts": xs,
                "r": np.ascontiguousarray(rs.T),
                "wlt": wlt_h,
                "w2t": w2t_h,
                "blb": blb_h,
            }
        )

    nc = build_nc(cfg, repeat=repeat, nq=nq, skip_gather=skip_gather, **bopts)
    res = bass_utils.run_bass_kernel_spmd(
        nc, in_maps, core_ids=list(range(n_cores)), trace=trace
    )
    out = np.concatenate(
        [res.results[c]["out"][:npc] for c in range(n_cores)], axis=0
    )
    out = np.ascontiguousarray(out, dtype=np.float32)
    # exact fix for cnt==0 nodes (device used W2 = Wr - Wl for all nodes)
    zero_nodes = np.nonzero(cnt_all == 0)[0]
    if len(zero_nodes):
        out[zero_nodes] = (
            x[zero_nodes] @ np.asarray(Wr, np.float32).T
            + np.asarray(bl, np.float32)[None, :]
        )
    return out, res


class Runner:
    """Jit the compiled Bass program once; support repeated timed runs.

    Mirrors bass2jax.run_bass_via_pjrt's multi-core path, but keeps the
    jitted callable and pre-placed device inputs so subsequent calls measure
    device execution without retrace/recompile or H2D of the big tensors.
    """

    def __init__(self, nc, in_maps, n_cores):
        import jax
        import jax.numpy as jnp
        from jax.sharding import Mesh, NamedSharding, PartitionSpec
        from jax.experimental.shard_map import shard_map

        from concourse import bass2jax as B2J
        from concourse import mybir as _mb

        B2J.install_neuronx_cc_hook()
        self.jax = jax
        partition_name = (
            nc.partition_id_tensor.name if nc.partition_id_tensor else None
        )
        in_names, out_names, out_avals, zero_outs = [], [], [], []
        for alloc in nc.m.functions[0].allocations:
            if not isinstance(alloc, _mb.MemoryLocationSet):
                continue
            name = alloc.memorylocations[0].name
            if alloc.kind == "ExternalInput":
                if name != partition_name:
                    in_names.append(name)
            elif alloc.kind == "ExternalOutput":
                shape = tuple(alloc.tensor_shape)
                dtype = _mb.dt.np(alloc.dtype)
                out_names.append(name)
                out_avals.append(jax.core.ShapedArray(shape, dtype))
                zero_outs.append(np.zeros(shape, dtype))
        n_params = len(in_names)
        all_in_names = list(in_names) + out_names
        if partition_name is not None:
            all_in_names.append(partition_name)
        donate = tuple(range(n_params, n_params + len(out_avals)))

        def _body(*args):
            operands = list(args)
            if partition_name is not None:
                operands.append(B2J.partition_id_tensor())
            outs = B2J._bass_exec_p.bind(
                *operands,
                out_avals=tuple(out_avals),
                in_names=tuple(all_in_names),
                out_names=tuple(out_names),
                lowering_input_output_aliases=(),
                sim_require_finite=True,
                sim_require_nnan=True,
                nc=nc,
            )
            return tuple(outs)

        devices = jax.devices()[:n_cores]
        mesh = Mesh(np.asarray(devices), ("core",))
        self.mesh = mesh
        spec = PartitionSpec("core")
        in_specs = (spec,) * (n_params + len(out_avals))
        out_specs = (spec,) * len(out_names)
        self.fn = jax.jit(
            shard_map(
                _body, mesh=mesh, in_specs=in_specs, out_specs=out_specs,
                check_rep=False,
            ),
            donate_argnums=donate,
            keep_unused=True,
        )
        sharding = NamedSharding(mesh, spec)
        concat_in = [
            np.concatenate([np.asarray(m[name]) for m in in_maps], axis=0)
            for name in in_names
        ]
        self.dev_in = [jax.device_put(a, sharding) for a in concat_in]
        self.zero_outs = zero_outs
        self.sharding = sharding
        self.out_names = out_names
        self.out_avals = out_avals
        self.n_cores = n_cores

    def _zeros(self):
        return [
            self.jax.device_put(
                np.zeros((self.n_cores * z.shape[0], *z.shape[1:]), z.dtype),
                self.sharding,
            )
            for z in self.zero_outs
        ]

    def run(self):
        outs = self.fn(*self.dev_in, *self._zeros())
        self.jax.block_until_ready(outs)
        return outs

    def timed(self, iters=20):
        import time

        zero_sets = [self._zeros() for _ in range(iters)]
        self.jax.block_until_ready(zero_sets)
        outs = None
        times = []
        for i in range(iters):
            t0 = time.perf_counter()
            outs = self.fn(*self.dev_in, *zero_sets[i])
            self.jax.block_until_ready(outs)
            times.append(time.perf_counter() - t0)
        return outs, times

    def results(self, outs):
        res = []
        for c in range(self.n_cores):
            res.append(
                {
                    name: np.asarray(outs[i]).reshape(
                        self.n_cores, *self.out_avals[i].shape
                    )[c]
                    for i, name in enumerate(self.out_names)
                }
            )
        return res


def make_runner(x, edge_index, Wl, bl, Wr, n_cores=8, group=GROUP, repeat=1,
                nq=NQ, **opts):
    """Build host data + compiled program + Runner (for timing loops)."""
    x = np.asarray(x, dtype=np.float32)
    saved = {}
    orig = bass_utils.run_bass_kernel_spmd

    def capture(nc, in_maps, core_ids, trace=False):
        saved["nc"], saved["in_maps"] = nc, in_maps
        raise _Captured()

    class _Captured(Exception):
        pass

    bass_utils.run_bass_kernel_spmd = capture
    try:
        run_graph(x, edge_index, Wl, bl, Wr, n_cores=n_cores, group=group,
                  repeat=repeat, nq=nq, **opts)
    except _Captured:
        pass
    finally:
        bass_utils.run_bass_kernel_spmd = orig
    return Runner(saved["nc"], saved["in_maps"], n_cores), saved


def kernel(**inputs) -> np.ndarray:
    out, _ = run_graph(
        inputs["x"],
        inputs["edge_index"],
        inputs["Wl"],
        inputs["bl"],
        inputs["Wr"],
        n_cores=8,
    )
    return out
